# revision 16
# baseline (speedup 1.0000x reference)
"""Trainium2 Bass kernel for nn_LogitGenerator (contrastive logits loss).

Reference computation:
    proj = x @ W + b                      # [16,1500,768] @ [768,256]
    pf = proj.reshape(-1, 256)
    for mask in (mask_m, mask_u):         # two [12000] int32 index sets
        px   = pf[mask]                   # gather frames
        pxn  = px / max(||px||, 1e-8)     # row-normalize
        embn = emb / max(||emb||, 1e-8)
        neg  = pxn @ embn.T               # [12000, 500] cosines
        pos  = neg[f, label[mask][f]]     # positive logit
        out  = [pos/T, where(c==tgt, -inf, neg/T)]   # [12000, 501], T=0.1

Sharding: the 12000+12000 mask entries are split into 8 contiguous chunks of
1500+1500; core i receives the full x (replicated, cast to bf16) plus its 3000
mask indices. No collectives.

Device pipeline per core (3072 padded rows):
    dma_gather(transpose=True): gathers the core's rows of x AND writes them
    transposed into SBUF as x^T bf16 tiles (6 chunks of 512 frames)
    -> mm1 (W bf16 stationary) -> proj^T evacuated as fp32r (+bias)
       with (proj+b)^2 computed from PSUM on ACT in the same pass
    -> per-frame sum-of-squares via PE ones-matmul -> rnorm = 1/max(||.||,eps)
    -> mm2 (proj^T slices stationary vs normalized emb^T, fp32r)
    -> scale rows by rnorm (cosines * 10) -> DMA out [3072, 500]

Host does only: input sharding/swizzles, bf16 cast of x, emb normalization
(0.1 MFLOP), and output assembly (pos-column extraction + -inf masking).
"""

from contextlib import ExitStack

import ml_dtypes
import numpy as np

import concourse.bass as bass
import concourse.tile as tile
from concourse import bacc, mybir
from concourse.bass_utils import run_bass_kernel_spmd

F32 = mybir.dt.float32
F32R = mybir.dt.float32r
BF16 = mybir.dt.bfloat16
I16 = mybir.dt.int16
I32 = mybir.dt.int32

N_CORES = 8
B, T, DE, DF, C = 16, 1500, 768, 256, 500
FTOT = B * T  # 24000
NMASK = 12000
CHUNK = NMASK // N_CORES  # 1500 rows of each mask per core
FPC = 3072  # padded rows per core (2*CHUNK -> 24 tiles of 128)
NT = FPC // 128  # 24 f-tiles
KE = DE // 128  # 6 e-blocks
ND = DF // 128  # 2 d-blocks
NFC = FPC // 512  # 6 f-chunks (also the gather granularity)
INV_TEMP = 10.0  # 1 / LOGIT_TEMP
EPS = 1e-8

_cache = {}


def _build_nc():
    nc = bacc.Bacc("TRN2", target_bir_lowering=False, debug=False, num_devices=N_CORES)
    x = nc.dram_tensor("x", [FTOT, DE], BF16, kind="ExternalInput").ap()
    idx = nc.dram_tensor("idx", [128, FPC // 16], I16, kind="ExternalInput").ap()
    w = nc.dram_tensor("w", [DE, DF], F32, kind="ExternalInput").ap()
    bias2 = nc.dram_tensor("bias2", [128, ND], F32, kind="ExternalInput").ap()
    embt = nc.dram_tensor("embt", [DF, C], F32, kind="ExternalInput").ap()
    out = nc.dram_tensor("out", [FPC, C], F32, kind="ExternalOutput").ap()

    with tile.TileContext(nc) as tc, ExitStack() as ctx:
        const = ctx.enter_context(tc.tile_pool(name="const", bufs=1))
        big = ctx.enter_context(tc.tile_pool(name="big", bufs=1))
        outp = ctx.enter_context(tc.tile_pool(name="outp", bufs=4))
        ps_sm = ctx.enter_context(tc.tile_pool(name="ps_sm", bufs=2, space="PSUM"))
        ps_mm = ctx.enter_context(tc.tile_pool(name="ps_mm", bufs=3, space="PSUM"))
        ps_g = ctx.enter_context(tc.tile_pool(name="ps_g", bufs=3, space="PSUM"))

        # ---- gathers first: Q7 descriptor emission (~4.7us/chunk) is the
        # critical path to the first matmul ----
        idx_sb = const.tile([128, FPC // 16], I16)
        nc.sync.dma_start(out=idx_sb[:], in_=idx[:])

        # x^T in chunk-major layout: [e%128, f-chunk, e//128, f%512] so each
        # dma_gather writes one contiguous [128, KE, 512] block.
        xT = big.tile([128, NFC, KE, 512], BF16)
        # chunk c covers frames [c*512, (c+1)*512): exactly mm1's f-chunk c.
        for c in range(NFC):
            nc.gpsimd.dma_gather(
                out_ap=xT[:, c, :, :],
                in_ap=x[:],
                idxs_ap=idx_sb[:, c * 32 : (c + 1) * 32],
                num_idxs=512,
                num_idxs_reg=512,
                elem_size=DE,
                transpose=True,
            )

        # ---- constants ----
        w_f32 = const.tile([128, KE, DF], F32)
        nc.sync.dma_start(out=w_f32[:], in_=w.rearrange("(k p) d -> p k d", p=128))
        w_sb = const.tile([128, KE, DF], BF16)
        nc.vector.tensor_copy(out=w_sb[:], in_=w_f32[:])
        b_sb = const.tile([128, ND], F32)
        nc.sync.dma_start(out=b_sb[:], in_=bias2[:])
        e_f32 = const.tile([128, ND, C], F32)
        nc.sync.dma_start(out=e_f32[:], in_=embt.rearrange("(k p) c -> p k c", p=128))
        e_sb = const.tile([128, ND, C], F32R)
        nc.vector.tensor_copy(out=e_sb[:], in_=e_f32[:])
        ones = const.tile([128, 1], F32)
        nc.vector.memset(ones[:], 1.0)
        ones_bf = const.tile([128, 1], BF16)
        nc.vector.memset(ones_bf[:], 1.0)
        eps2t = const.tile([128, 1], F32)
        nc.vector.memset(eps2t[:], EPS * EPS)

        projT = big.tile([128, ND, FPC], F32R)  # proj^T (bias added), rounded fp32r
        sq = big.tile([128, ND, FPC], BF16)  # (proj+b)^2, for row norms
        ssrow = big.tile([1, FPC], F32)  # ssum as a single-partition row
        rs = const.tile([128, NT], F32)  # max(||proj||, eps) per frame
        rnorm = const.tile([128, NT], F32)  # its reciprocal, col t <-> tile t

        # ---- phases B/C/D fused per 512-frame chunk so mm2/scale/store of
        # earlier chunks fill PE stalls while later gathers land ----
        for fc in range(NFC):
            sl = slice(fc * 512, (fc + 1) * 512)
            # mm1: proj^T = W^T @ x^T; evac fp32r (+bias); (proj+b)^2 on ACT
            for d in range(ND):
                ps = ps_mm.tile([128, 512], F32, tag="mm", name=f"ps_{fc}_{d}")
                for k in range(KE):
                    nc.tensor.matmul(
                        out=ps[:],
                        lhsT=w_sb[:, k, d * 128 : (d + 1) * 128],
                        rhs=xT[:, fc, k, :],
                        start=(k == 0),
                        stop=(k == KE - 1),
                    )
                nc.vector.tensor_scalar_add(
                    out=projT[:, d, sl], in0=ps[:], scalar1=b_sb[:, d : d + 1]
                )
                nc.scalar.activation(
                    out=sq[:, d, sl],
                    in_=ps[:],
                    func=mybir.ActivationFunctionType.Square,
                    bias=b_sb[:, d : d + 1],
                )
            # row sums of squares for this chunk: ones^T @ sq -> [1, 512]
            pr = ps_sm.tile([1, 512], F32, tag="sm", name=f"pr_{fc}")
            for d in range(ND):
                nc.tensor.matmul(
                    out=pr[:],
                    lhsT=ones_bf[:],
                    rhs=sq[:, d, sl],
                    start=(d == 0),
                    stop=(d == ND - 1),
                )
            nc.vector.tensor_copy(out=ssrow[:, sl], in_=pr[:])
            # per 128-frame tile: rnorm, mm2 cosines, scale, store
            for t in range(fc * 4, (fc + 1) * 4):
                pt = ps_sm.tile([128, 1], F32, tag="sm", name=f"pt_{t}")
                nc.tensor.matmul(
                    out=pt[:],
                    lhsT=ssrow[:, t * 128 : (t + 1) * 128],
                    rhs=ones[0:1, 0:1],
                    start=True,
                    stop=True,
                )
                # sqrt(ssum + eps^2) == max(||proj||, eps) up to the eps floor,
                # then per-tile reciprocal: no cross-tile barrier.
                nc.scalar.activation(
                    out=rs[:, t : t + 1],
                    in_=pt[:],
                    func=mybir.ActivationFunctionType.Sqrt,
                    bias=eps2t[:, 0:1],
                )
                nc.vector.reciprocal(out=rnorm[:, t : t + 1], in_=rs[:, t : t + 1])
                g = ps_g.tile([128, 512], F32, tag="g", name=f"g_{t}")
                for d in range(ND):
                    nc.tensor.matmul(
                        out=g[:, :C],
                        lhsT=projT[:, d, t * 128 : (t + 1) * 128],
                        rhs=e_sb[:, d, :],
                        start=(d == 0),
                        stop=(d == ND - 1),
                    )
                ot = outp.tile([128, C], F32)
                if t % 2 == 0:
                    nc.vector.tensor_scalar_mul(
                        out=ot[:], in0=g[:, :C], scalar1=rnorm[:, t : t + 1]
                    )
                else:
                    nc.scalar.activation(
                        out=ot[:],
                        in_=g[:, :C],
                        func=mybir.ActivationFunctionType.Copy,
                        scale=rnorm[:, t : t + 1],
                    )
                nc.sync.dma_start(out=out[t * 128 : (t + 1) * 128, :], in_=ot[:])

    nc.compile()
    return nc


def _get_nc():
    if "nc" not in _cache:
        _cache["nc"] = _build_nc()
    return _cache["nc"]


def kernel(x, label, mask_m, mask_u, W, b, emb):
    nc = _get_nc()

    x_bf = np.ascontiguousarray(
        np.asarray(x, np.float32).reshape(FTOT, DE).astype(ml_dtypes.bfloat16)
    )
    W = np.ascontiguousarray(np.asarray(W, np.float32))
    b = np.asarray(b, np.float32)
    emb = np.asarray(emb, np.float32)
    mask_m = np.asarray(mask_m, np.int32)
    mask_u = np.asarray(mask_u, np.int32)
    label = np.asarray(label, np.int32)

    en = emb / np.maximum(np.linalg.norm(emb, axis=-1, keepdims=True), EPS)
    embt = np.ascontiguousarray(en.T * np.float32(INV_TEMP)).astype(np.float32)
    bias2 = np.ascontiguousarray(b.reshape(ND, 128).T)

    in_maps = []
    for i in range(N_CORES):
        idx = np.zeros(FPC, np.int16)
        idx[:CHUNK] = mask_m[i * CHUNK : (i + 1) * CHUNK]
        idx[CHUNK : 2 * CHUNK] = mask_u[i * CHUNK : (i + 1) * CHUNK]
        # dma_gather layout: index j lives at partition j%16, column j//16,
        # with the 16-partition block replicated down all 128 partitions.
        blk = idx.reshape(FPC // 16, 16).T  # [16, 192]
        idx_sw = np.ascontiguousarray(np.tile(blk, (8, 1)))  # [128, 192]
        in_maps.append(
            {"x": x_bf, "idx": idx_sw, "w": W, "bias2": bias2, "embt": embt}
        )

    res = run_bass_kernel_spmd(nc, in_maps, core_ids=list(range(N_CORES)))
    _cache["last_results"] = res
    outs = [r["out"] for r in res.results]

    cos_m = np.concatenate([o[:CHUNK] for o in outs], axis=0)  # [12000, 500] = 10*cos
    cos_u = np.concatenate([o[CHUNK : 2 * CHUNK] for o in outs], axis=0)

    lf = label.reshape(-1)
    # duplicate-row groups of emb: replicate reference's exact-equality mask
    _, grp = np.unique(emb, axis=0, return_inverse=True)

    def assemble(cos, mask):
        tgt = lf[mask]
        n = cos.shape[0]
        res = np.empty((n, C + 1), np.float32)
        res[:, 0] = cos[np.arange(n), tgt]
        res[:, 1:] = cos
        pos_mask = grp[tgt][:, None] == grp[None, :]
        res[:, 1:][pos_mask] = -np.inf
        return res

    return assemble(cos_m, mask_m), assemble(cos_u, mask_u)
